# revision 7
# baseline (speedup 1.0000x reference)
"""Trainium2 Bass kernel for CausalAttentionSortNet bucket-scoring.

Math (see reference): only `k` feeds the output. For each merged batch*head
slice, the cumulative-average of k is sampled at bucket starts (every 128th
row), which reduces to per-chunk sums + a strictly-triangular prefix matmul.
The rest is tiny per-bucket sort projections and a 64x65 masked softmax.

Sharding: data-parallel over the merged (batch*heads)=32 axis across 8 cores,
4 slices per core, processed as 2 pairs of 2 slices; a pair fills the
128-partition dim as partition=(slice_in_pair, chunk), free=(pair, row, dim)
so every partition's k data is contiguous 32KB HBM runs (DMA saturates all
16 engines at ~350 GB/s).

`q` (half of all input bytes) is never read by the reference computation, so
it is not even transferred to the device.

DMA-instruction budget: the hardware exposes ~12 DMA completion semaphores;
a 17-instruction version stalled the bulk queue 12us on semaphore reuse
against the slow small-packet constant stream. This version issues exactly
8 DMAs: 1 packed-constant + 6 bulk (both pairs per instruction) + 1 output.
The chunk first-rows are NOT a separate (256-descriptor, half-rate) DMA:
they arrive inside the first bulk tile, whose in-place fold targets its
upper half so row 0 survives for the F-term matmuls.

Per-chunk reduction: halving cascade over sub-tiles of (64,32,16,8,4,4)
rows; the running partial X is folded in half (contiguous tensor_add, full
DVE rate) to the next sub-tile's size and added into it, so the big adds
overlap the bulk DMA of later tiles, and after the last 4-row tile lands
only a 256-element add + a small strided reduce remain. The chunk-sums ->
scaled-prefix conversion is one matmul per pair against the tril*scale
constant, closing a PSUM group the F-term matmul opened mid-stream.
"""

from contextlib import ExitStack

import numpy as np

import concourse.bacc as bacc
import concourse.mybir as mybir
import concourse.tile as tile
from concourse import bass_utils

# Problem constants (hardcoded per contract; kernel.py must be self-contained).
B, HEADS, BUCKETS, DIM, DIM_SORT, T = 4, 8, 64, 64, 8, 8192
BH = B * HEADS            # 32 merged batch*head slices
NCORES = 8
BHC = BH // NCORES        # 4 slices per core
NPAIR = BHC // 2          # 2 pairs per core
CHUNK = T // BUCKETS      # 128 rows per bucket
NEG = -1.0e30             # softmax mask value (underflows exp to exactly 0)
FP = mybir.dt.float32

# packed-constant column offsets
NC128 = 128 * 5 + 2
NC64 = 4 * 104
NC104 = 2 * 128
NCALL = NC128 + NC104

# first GP_N cascade ops of the first-landing pair run on GpSimd; the rest
# (including everything near the tail) on the faster DVE
GP_N = 3

TRACE = False  # set by test.py for profiling runs
TRACE_KWARGS = {}  # extra run_bass_kernel_spmd kwargs for profiling runs
LAST_RESULTS = None  # BassKernelResults of the most recent run

_PROG_CACHE = {}


def _cascade_sizes(chunk):
    # halve down to 4, then one more 4: (64, 32, 16, 8, 4, 4) for chunk=128
    assert chunk >= 8
    sizes = []
    r = chunk
    while r > 4:
        r //= 2
        sizes.append(r)
    sizes.append(4)
    assert sum(sizes) == chunk, (sizes, chunk)
    return sizes


def _build_program(t_seq=T, enable_asserts=False, debug_taps=False):
    chunk = t_seq // BUCKETS
    sizes = _cascade_sizes(chunk)
    nsub = len(sizes)

    nc = bacc.Bacc(
        "TRN2",
        target_bir_lowering=False,
        debug=False,
        enable_asserts=enable_asserts,
        num_devices=NCORES,
    )

    def din(name, shape):
        return nc.dram_tensor(name, shape, FP, kind="ExternalInput").ap()

    kin = din("kin", (BHC, t_seq, DIM))
    # packed constants, two DMAs:
    # cpack cols 0:642     c128 = [lmat_s | idents | ident | amask_b | mmask_b | mask0_b]
    # cpack cols 642:898   c104 = per pair (104, 128) cq/ck blocks (rows 104:128 zero)
    # c64 (64, 416)        [wqk_pt_p0 | wqk_pt_p1 | wqk_ft_p0 | wqk_ft_p1]
    cpack = din("cpack", (128, NCALL))
    c64 = din("c64", (64, NC64))
    rout = nc.dram_tensor(
        "rout", (BHC, BUCKETS, BUCKETS + 1), FP, kind="ExternalOutput"
    ).ap()
    taps = {}
    if debug_taps:
        taps["pt"] = nc.dram_tensor("tap_pt", (128, 128), FP, kind="ExternalOutput").ap()
        taps["ft"] = nc.dram_tensor("tap_ft", (128, 128), FP, kind="ExternalOutput").ap()
        taps["par"] = nc.dram_tensor("tap_par", (128, 128), FP, kind="ExternalOutput").ap()

    X = mybir.AxisListType.X
    Exp = mybir.ActivationFunctionType.Exp
    MULT = mybir.AluOpType.mult

    with tile.TileContext(nc) as tc:
        with ExitStack() as ctx:
            singles = ctx.enter_context(tc.tile_pool(name="singles", bufs=1))
            kpools = [
                ctx.enter_context(tc.tile_pool(name=f"kpool{s}", bufs=1))
                for s in range(nsub)
            ]
            small = ctx.enter_context(tc.tile_pool(name="small", bufs=2))
            pp = ctx.enter_context(tc.tile_pool(name="pp", bufs=1, space="PSUM"))

            cp_sb = singles.tile([128, NCALL], FP, tag="cpack")
            nc.scalar.dma_start(cp_sb[:], cpack)
            c64_sb = singles.tile([64, NC64], FP, tag="c64")
            nc.scalar.dma_start(c64_sb[:], c64)

            # ---- bulk k sub-tile DMAs, single queue, both pairs per
            # instruction (2 contiguous rows*256B runs per partition)
            ksrc = kin.rearrange(
                "(p b) (c r) d -> (b c) p r d", p=NPAIR, r=chunk
            )
            kts = []
            r0 = 0
            for s, rs in enumerate(sizes):
                kt = kpools[s].tile([128, NPAIR, rs, DIM], FP, tag=f"kt{s}")
                nc.sync.dma_start(kt[:], ksrc[:, :, r0 : r0 + rs, :])
                kts.append(kt)
                r0 += rs

            lmat_s = cp_sb[:, 0:128]
            idents = cp_sb[:, 128:256]
            ident = cp_sb[:, 256:384]
            # amask (cols 384:512) is consumed directly by the R-group matmul
            mmask_b = cp_sb[:, 512:640].rearrange("q (p j) -> q p j", p=2)
            mask0_b = cp_sb[:, 640:642]

            # ---- PSUM groups, one bank per (pair, tensor): FT_p is F
            # transposed; PT_p is opened by the F*diag(s) seed and closed by
            # that pair's chunk-sum prefix matmul. F = row 0 of the first
            # bulk tile. Separate banks let pair 0's whole epilogue run while
            # pair 1 is still streaming.
            PT_ps = [
                pp.tile([64, 128], FP, tag=f"PT{p}", name=f"PT_ps{p}")
                for p in range(NPAIR)
            ]
            FT_ps = [
                pp.tile([64, 128], FP, tag=f"FT{p}", name=f"FT_ps{p}")
                for p in range(NPAIR)
            ]
            for p in range(NPAIR):
                nc.tensor.matmul(
                    FT_ps[p][:],
                    lhsT=kts[0][:, p, 0, :],
                    rhs=ident,
                    start=True,
                    stop=True,
                )
                nc.tensor.matmul(
                    PT_ps[p][:],
                    lhsT=kts[0][:, p, 0, :],
                    rhs=idents,
                    start=True,
                    stop=False,
                )

            # ---- halving-cascade chunk reduction, one chain per pair.
            # The first fold targets the tile's upper half so row 0 (the F
            # term) survives; later folds shrink in place toward row 0 of
            # their own tiles.
            par_both = singles.tile([128, NPAIR, DIM], FP, tag="par")
            for p in (1, 0):
                opi = 0

                def eng(p=p):
                    nonlocal opi
                    e = nc.gpsimd if (p == 1 and opi < GP_N) else nc.vector
                    opi += 1
                    return e

                h0 = sizes[0] // 2
                eng().tensor_add(
                    kts[0][:, p, h0 : sizes[0], :],
                    kts[0][:, p, h0 : sizes[0], :],
                    kts[0][:, p, 0:h0, :],
                )
                xt, xlo, xr = kts[0], h0, h0
                for s in range(1, nsub):
                    rs = sizes[s]
                    while xr > rs:
                        h = xr // 2
                        eng().tensor_add(
                            xt[:, p, xlo : xlo + h, :],
                            xt[:, p, xlo : xlo + h, :],
                            xt[:, p, xlo + h : xlo + xr, :],
                        )
                        xr = h
                    kt = kts[s]
                    eng().tensor_add(
                        kt[:, p, :, :], kt[:, p, :, :], xt[:, p, xlo : xlo + xr, :]
                    )
                    xt, xlo, xr = kt, 0, rs
                eng().reduce_sum(
                    par_both[:, p, :],
                    xt[:, p, :, :].rearrange("q r d -> q d r"),
                    axis=X,
                )
                nc.tensor.matmul(
                    PT_ps[p][:],
                    lhsT=par_both[:, p, :],
                    rhs=lmat_s,
                    start=False,
                    stop=True,
                )

            # ---- sort projections (per pair), batched softmax (both pairs)
            PT_sb = [
                small.tile([64, 128], FP, tag=f"PTs{p}", name=f"PT_sb{p}")
                for p in range(NPAIR)
            ]
            FT_sb = [
                small.tile([64, 128], FP, tag=f"FTs{p}", name=f"FT_sb{p}")
                for p in range(NPAIR)
            ]
            for p in range(NPAIR):
                nc.scalar.copy(FT_sb[p][:], FT_ps[p][:])
                nc.scalar.copy(PT_sb[p][:], PT_ps[p][:])
            if debug_taps:
                for p in range(NPAIR):
                    nc.sync.dma_start(taps["pt"][64 * p : 64 * p + 64], PT_sb[p][:])
                    nc.sync.dma_start(taps["ft"][64 * p : 64 * p + 64], FT_sb[p][:])
                nc.sync.dma_start(taps["par"], par_both[:])

            # SKQ rows: 0:40 sort-q blocks (b0 at 0:8, b1 at 32:40),
            #           64:104 sort-k blocks (b0 at 64:72, b1 at 96:104);
            # one PSUM bank per pair: each holds a long-open accumulation group
            # opened by the constant-term matmul (ready at kernel start) and
            # closed by the PT-part matmul (the only one on the critical tail)
            C104O = NC128
            SQs = []
            RKs = []
            for p in range(NPAIR):
                sk_ps_t = pp.tile([104, 128], FP, tag=f"SKQ{p}")
                sk_ps = sk_ps_t[:]
                nc.tensor.matmul(
                    sk_ps,
                    lhsT=ident[0:104, 0:104],
                    rhs=cp_sb[0:104, C104O + 128 * p : C104O + 128 * p + 128],
                    start=True,
                    stop=False,
                    skip_group_check=True,
                )
                nc.tensor.matmul(
                    sk_ps,
                    lhsT=c64_sb[:, 208 + 104 * p : 312 + 104 * p],
                    rhs=FT_sb[p][:],
                    start=False,
                    stop=False,
                    skip_group_check=True,
                )
                nc.tensor.matmul(
                    sk_ps,
                    lhsT=c64_sb[:, 104 * p : 104 * p + 104],
                    rhs=PT_sb[p][:],
                    start=False,
                    stop=True,
                    skip_group_check=True,
                )
                sq_sb = small.tile([40, 128], FP, tag=f"SQ{p}")
                nc.scalar.copy(sq_sb[:], sk_ps[0:40, :])
                rk_sb = small.tile([40, 128], FP, tag=f"RK{p}")
                nc.vector.tensor_copy(rk_sb[:], sk_ps[64:104, :])
                SQs.append(sq_sb)
                RKs.append(rk_sb)

            # R group: opened early by an identity-weighted matmul that seeds
            # the bank with the additive causal mask; the four sq.sk matmuls
            # then accumulate into their quadrants, so the masked logits sit
            # in PSUM with no extra elementwise pass
            R_ps = pp.tile([128, 128], FP, tag="R")
            nc.tensor.matmul(
                R_ps[:],
                lhsT=ident,
                rhs=cp_sb[:, 384:512],
                start=True,
                stop=False,
                skip_group_check=True,
            )
            for p in range(NPAIR):
                nc.tensor.matmul(
                    R_ps[0:64, 64 * p : 64 * p + 64],
                    lhsT=SQs[p][0:8, 0:64],
                    rhs=RKs[p][0:8, 0:64],
                    start=False,
                    stop=False,
                    skip_group_check=True,
                )
                nc.tensor.matmul(
                    R_ps[64:128, 64 * p : 64 * p + 64],
                    lhsT=SQs[p][32:40, 64:128],
                    rhs=RKs[p][32:40, 64:128],
                    start=False,
                    stop=p == NPAIR - 1,
                    skip_group_check=True,
                )

            # masked softmax over 65 logits (implicit zero-logit column 0),
            # both pairs batched along the free axis: cols = (pair, j)
            Rm = R_ps[:].rearrange("q (p j) -> q p j", p=2)
            mx = small.tile([128, 2], FP, tag="mx")
            nc.vector.reduce_max(mx[:], Rm, axis=X)
            negm = small.tile([128, 2], FP, tag="negm")
            nc.vector.tensor_scalar(
                negm[:], mx[:], 0.0, -1.0,
                op0=mybir.AluOpType.max, op1=MULT,
            )
            e0 = small.tile([128, 2], FP, tag="e0")
            nc.scalar.activation(e0[:], negm[:], Exp)
            e_sb = small.tile([128, 2, 64], FP, tag="e")
            for p in range(NPAIR):
                nc.scalar.activation(
                    e_sb[:, p, :], R_ps[:, 64 * p : 64 * p + 64], Exp,
                    bias=negm[:, p : p + 1], scale=1.0,
                )
            s1 = small.tile([128, 2], FP, tag="s1")
            nc.vector.reduce_sum(s1[:], e_sb[:], axis=X)
            den = small.tile([128, 2], FP, tag="den")
            nc.vector.tensor_add(den[:], s1[:], e0[:])
            rin = small.tile([128, 2], FP, tag="rin")
            nc.vector.reciprocal(rin[:], den[:])
            outt = small.tile([128, 2, BUCKETS + 1], FP, tag="outt")
            for p in range(NPAIR):
                # outt = (e * 1/den) * tril-mask, fused
                nc.vector.scalar_tensor_tensor(
                    outt[:, p, 1:],
                    e_sb[:, p, :],
                    rin[:, p : p + 1],
                    mmask_b[:, p, :],
                    op0=MULT,
                    op1=MULT,
                )
            t0 = small.tile([128, 2], FP, tag="t0")
            nc.vector.tensor_mul(t0[:], e0[:], mask0_b)
            nc.vector.tensor_mul(outt[:, :, 0], t0[:], rin[:])
            nc.sync.dma_start(
                rout.rearrange("(p b) i c -> (b i) p c", p=2), outt[:]
            )

    nc.compile()
    return nc


def _get_program(t_seq=T, enable_asserts=False):
    key = (t_seq, enable_asserts)
    if key not in _PROG_CACHE:
        _PROG_CACHE[key] = _build_program(t_seq, enable_asserts=enable_asserts)
    return _PROG_CACHE[key]


def _host_constants(core, q_pos_emb, k_pos_emb, Wsq, Wsk, chunk=CHUNK):
    """Single packed per-core constant tensor."""
    f32 = np.float32
    j = np.arange(64, dtype=np.float64)
    s = (1.0 / (chunk * j + 1.0)).astype(f32)  # per-bucket cumavg scale

    tri = np.triu(np.ones((64, 64), f32), k=1)  # [c, j] = 1 iff c < j
    tri_s = tri * s[None, :]
    lmat_s = np.zeros((128, 128), f32)
    lmat_s[0:64, 0:64] = tri_s
    lmat_s[64:128, 64:128] = tri_s
    idents = np.zeros((128, 128), f32)
    idents[np.arange(128), np.arange(128)] = np.concatenate([s, s])
    ident = np.eye(128, dtype=f32)

    rows = np.arange(64)[:, None]
    cols = np.arange(64)[None, :]
    am = np.where(cols < rows, 0.0, NEG).astype(f32)       # softmax additive mask
    mm = (cols <= rows - 2).astype(f32)                    # output tril(-1) mask
    amask_b = np.concatenate([am, am], axis=1)
    amask_b = np.concatenate([amask_b, amask_b], axis=0)
    mmask_b = np.concatenate([mm, mm], axis=1)
    mmask_b = np.concatenate([mmask_b, mmask_b], axis=0)
    m0 = (np.arange(64) > 0).astype(f32).reshape(64, 1)
    mask0_b = np.concatenate([np.concatenate([m0, m0], 1)] * 2, 0)

    c128 = np.concatenate([lmat_s, idents, ident, amask_b, mmask_b, mask0_b], axis=1)

    wq_pt = np.zeros((2, 64, 104), f32)   # [pair][d][sq 0:40 | sk 64:104]
    wq_ft = np.zeros((2, 64, 104), f32)
    cblk = np.zeros((2, 104, 128), f32)   # [pair][skq-row][(b, j)]
    for p in range(NPAIR):
        for b in range(2):
            bh = core * BHC + 2 * p + b
            h = bh % HEADS
            r0 = 32 * b
            wq_pt[p, :, r0 : r0 + 8] = Wsq[0, h, 0:64, :]
            wq_pt[p, :, 64 + r0 : 64 + r0 + 8] = Wsk[0, h, 0:64, :]
            wq_ft[p, :, r0 : r0 + 8] = Wsq[0, h, 64:128, :]
            wq_ft[p, :, 64 + r0 : 64 + r0 + 8] = Wsk[0, h, 64:128, :]
            cq = q_pos_emb[0, h] @ Wsq[0, h, 128:192, :]  # (64, 8)
            ck = k_pos_emb[0, h] @ Wsk[0, h, 128:192, :]
            cblk[p, r0 : r0 + 8, 64 * b : 64 * b + 64] = cq.T
            cblk[p, 64 + r0 : 64 + r0 + 8, 64 * b : 64 * b + 64] = ck.T

    c64 = np.concatenate([wq_pt[0], wq_pt[1], wq_ft[0], wq_ft[1]], axis=1)
    c104 = np.concatenate([cblk[0], cblk[1]], axis=1)
    c104 = np.concatenate([c104, np.zeros((24, NC104), f32)], axis=0)
    cpack = np.concatenate([c128, c104], axis=1)
    assert cpack.shape == (128, NCALL), cpack.shape
    assert c64.shape == (64, NC64), c64.shape
    return {"cpack": cpack, "c64": c64}


def _run(k, q_pos_emb, k_pos_emb, Wsq, Wsk, trace=False, t_seq=T):
    nc = _get_program(t_seq)
    in_maps = []
    for core in range(NCORES):
        cm = _host_constants(
            core, q_pos_emb, k_pos_emb, Wsq, Wsk, chunk=t_seq // BUCKETS
        )
        cm["kin"] = np.ascontiguousarray(k[core * BHC : (core + 1) * BHC])
        in_maps.append(cm)
    res = bass_utils.run_bass_kernel_spmd(
        nc,
        in_maps,
        core_ids=list(range(NCORES)),
        trace=trace,
        **(TRACE_KWARGS if trace else {}),
    )
    global LAST_RESULTS
    LAST_RESULTS = res
    out = np.concatenate([r["rout"] for r in res.results], axis=0)
    return out, res


def kernel(**inputs):
    k = np.asarray(inputs["k"], np.float32)
    q_pos_emb = np.asarray(inputs["q_pos_emb"], np.float32)
    k_pos_emb = np.asarray(inputs["k_pos_emb"], np.float32)
    Wsq = np.asarray(inputs["Wsq"], np.float32)
    Wsk = np.asarray(inputs["Wsk"], np.float32)
    out, _ = _run(k, q_pos_emb, k_pos_emb, Wsq, Wsk, trace=TRACE)
    return out


# revision 8
# speedup vs baseline: 1.1348x; 1.1348x over previous
"""Trainium2 Bass kernel for CausalAttentionSortNet bucket-scoring.

Math (see reference): only `k` feeds the output. For each merged batch*head
slice, the cumulative-average of k is sampled at bucket starts (every 128th
row), which reduces to per-chunk sums + a strictly-triangular prefix matmul.
The rest is tiny per-bucket sort projections and a 64x65 masked softmax.

Sharding: data-parallel over the merged (batch*heads)=32 axis across 8 cores,
4 slices per core, processed as 2 pairs of 2 slices; a pair fills the
128-partition dim as partition=(slice_in_pair, chunk), free=(pair, row, dim)
so every partition's k data is contiguous 32KB HBM runs (DMA saturates all
16 engines at ~350 GB/s).

`q` (half of all input bytes) is never read by the reference computation, so
it is not even transferred to the device.

DMA-instruction budget: the hardware exposes ~12 DMA completion semaphores,
so a DMA instruction >=12 positions later reuses an earlier one's semaphore
and its issue blocks until that user completes. All constants ship in two
early-completing DMAs and the bulk tiles are uniform, so every reuse target
is long done by the time its semaphore is recycled (a version that put a
slow small-packet constant DMA in the reuse chain stalled the bulk queue
12us). Chunk first-rows are not a separate DMA: they arrive inside each
pair's first bulk tile, whose fold targets the tile's upper half so row 0
survives for the F-term matmuls.

Per-chunk reduction: each pair's rows stream as sub-tiles of
(16x7, 8, 4, 4) rows. Mid-stream, SBUF port contention caps DVE at
~1.8ns/elem and GpSimd at ~2.6ns/elem (vs 1.04/2.0 idle), so each sub-tile
gets an INDEPENDENT halving-fold chain (contiguous tensor_adds down to one
row -> its own partial-sum slot) and the chains are statically balanced
across both engines; a long serial cascade on one engine trailed the
stream by 12us. The PE (otherwise idle) folds every partial into the
scaled-prefix via one matmul per sub-tile against the tril*scale constant,
accumulating in that pair's PSUM bank, opened by the F*diag(s) seed and
closed by the last sub-tile's matmul. Small sub-tiles stream last so the
post-stream tail is two ~0.3us fold chains plus the epilogue.
"""

from contextlib import ExitStack

import numpy as np

import concourse.bacc as bacc
import concourse.mybir as mybir
import concourse.tile as tile
from concourse import bass_utils

# Problem constants (hardcoded per contract; kernel.py must be self-contained).
B, HEADS, BUCKETS, DIM, DIM_SORT, T = 4, 8, 64, 64, 8, 8192
BH = B * HEADS            # 32 merged batch*head slices
NCORES = 8
BHC = BH // NCORES        # 4 slices per core
NPAIR = BHC // 2          # 2 pairs per core
CHUNK = T // BUCKETS      # 128 rows per bucket
NEG = -1.0e30             # softmax mask value (underflows exp to exactly 0)
FP = mybir.dt.float32

# packed-constant column offsets
NC128 = 128 * 5 + 2
NC64 = 4 * 104
NC104 = 2 * 128
NCALL = NC128 + NC104

# pair-1 fold chains for sub-tiles [0, GP_CHAINS) run on GpSimd; all other
# chains (including every chain near the tail) on the faster DVE
GP_CHAINS = 6

TRACE = False  # set by test.py for profiling runs
TRACE_KWARGS = {}  # extra run_bass_kernel_spmd kwargs for profiling runs
LAST_RESULTS = None  # BassKernelResults of the most recent run

_PROG_CACHE = {}


def _cascade_sizes(chunk):
    # uniform mid-size tiles, small ones last: (16x7, 8, 4, 4) for chunk=128
    assert chunk == 128, "sub-tile schedule is tuned for chunk=128"
    sizes = [16] * 7 + [8, 4, 4]
    assert sum(sizes) == chunk, (sizes, chunk)
    return sizes


def _build_program(t_seq=T, enable_asserts=False, debug_taps=False):
    chunk = t_seq // BUCKETS
    sizes = _cascade_sizes(chunk)
    nsub = len(sizes)

    nc = bacc.Bacc(
        "TRN2",
        target_bir_lowering=False,
        debug=False,
        enable_asserts=enable_asserts,
        num_devices=NCORES,
    )

    def din(name, shape):
        return nc.dram_tensor(name, shape, FP, kind="ExternalInput").ap()

    kin = din("kin", (BHC, t_seq, DIM))
    # packed constants, two DMAs:
    # cpack cols 0:642     c128 = [lmat_s | idents | ident | amask_b | mmask_b | mask0_b]
    # cpack cols 642:898   c104 = per pair (104, 128) cq/ck blocks (rows 104:128 zero)
    # c64 (64, 416)        [wqk_pt_p0 | wqk_pt_p1 | wqk_ft_p0 | wqk_ft_p1]
    cpack = din("cpack", (128, NCALL))
    c64 = din("c64", (64, NC64))
    rout = nc.dram_tensor(
        "rout", (BHC, BUCKETS, BUCKETS + 1), FP, kind="ExternalOutput"
    ).ap()
    taps = {}
    if debug_taps:
        taps["pt"] = nc.dram_tensor("tap_pt", (128, 128), FP, kind="ExternalOutput").ap()
        taps["ft"] = nc.dram_tensor("tap_ft", (128, 128), FP, kind="ExternalOutput").ap()
        taps["par"] = nc.dram_tensor("tap_par", (128, 128), FP, kind="ExternalOutput").ap()

    X = mybir.AxisListType.X
    Exp = mybir.ActivationFunctionType.Exp
    MULT = mybir.AluOpType.mult

    with tile.TileContext(nc) as tc:
        with ExitStack() as ctx:
            singles = ctx.enter_context(tc.tile_pool(name="singles", bufs=1))
            kpools = [
                ctx.enter_context(tc.tile_pool(name=f"kpool{s}", bufs=2))
                for s in range(nsub)
            ]
            parp = ctx.enter_context(tc.tile_pool(name="parp", bufs=nsub))
            small = ctx.enter_context(tc.tile_pool(name="small", bufs=2))
            pp = ctx.enter_context(tc.tile_pool(name="pp", bufs=1, space="PSUM"))

            cp_sb = singles.tile([128, NCALL], FP, tag="cpack")
            nc.scalar.dma_start(cp_sb[:], cpack)
            c64_sb = singles.tile([64, NC64], FP, tag="c64")
            nc.scalar.dma_start(c64_sb[:], c64)

            # ---- bulk k sub-tile DMAs, single queue, pair 1 leading so
            # its GpSimd chains start first (contiguous rows*256B runs per
            # partition)
            ksrcs = [
                kin[2 * p : 2 * p + 2].rearrange("b (c r) d -> (b c) r d", r=chunk)
                for p in range(NPAIR)
            ]
            kts = {}
            r0 = 0
            for s, rs in enumerate(sizes):
                for p in (1, 0):
                    kt = kpools[s].tile(
                        [128, rs, DIM], FP, tag=f"kt{s}", name=f"kt{s}_{p}"
                    )
                    nc.sync.dma_start(kt[:], ksrcs[p][:, r0 : r0 + rs, :])
                    kts[(p, s)] = kt
                r0 += rs

            lmat_s = cp_sb[:, 0:128]
            idents = cp_sb[:, 128:256]
            ident = cp_sb[:, 256:384]
            # amask (cols 384:512) is consumed directly by the R-group matmul
            mmask_b = cp_sb[:, 512:640].rearrange("q (p j) -> q p j", p=2)
            mask0_b = cp_sb[:, 640:642]

            # ---- PSUM groups, one bank per (pair, tensor): FT_p is F
            # transposed; PT_p is opened by the F*diag(s) seed and closed by
            # that pair's chunk-sum prefix matmul. F = row 0 of the first
            # bulk tile. Separate banks let pair 0's whole epilogue run while
            # pair 1 is still streaming.
            PT_ps = [
                pp.tile([64, 128], FP, tag=f"PT{p}", name=f"PT_ps{p}")
                for p in range(NPAIR)
            ]
            FT_ps = [
                pp.tile([64, 128], FP, tag=f"FT{p}", name=f"FT_ps{p}")
                for p in range(NPAIR)
            ]
            for p in range(NPAIR):
                nc.tensor.matmul(
                    FT_ps[p][:],
                    lhsT=kts[(p, 0)][:, 0, :],
                    rhs=ident,
                    start=True,
                    stop=True,
                )
                nc.tensor.matmul(
                    PT_ps[p][:],
                    lhsT=kts[(p, 0)][:, 0, :],
                    rhs=idents,
                    start=True,
                    stop=False,
                )

            # ---- per-sub-tile fold chains + per-sub-tile prefix matmuls.
            # Each (pair, sub-tile) folds independently down to one row (the
            # first fold targets the upper half so row 0 survives in tile 0),
            # writing its own partial-sum slot; the PE folds every partial
            # into the pair's scaled-prefix PSUM bank as it appears.
            pars = [
                parp.tile([128, NPAIR, DIM], FP, tag=f"par{s}", name=f"par{s}")
                for s in range(nsub)
            ]
            for s, rs in enumerate(sizes):
                for p in (1, 0):
                    t = kts[(p, s)]
                    e = nc.gpsimd if (p == 1 and s < GP_CHAINS) else nc.vector
                    h = rs // 2
                    e.tensor_add(t[:, h:rs, :], t[:, h:rs, :], t[:, 0:h, :])
                    lo, xr = h, h
                    while xr > 2:
                        hh = xr // 2
                        e.tensor_add(
                            t[:, lo : lo + hh, :],
                            t[:, lo : lo + hh, :],
                            t[:, lo + hh : lo + xr, :],
                        )
                        xr = hh
                    e.tensor_add(
                        pars[s][:, p, :], t[:, lo, :], t[:, lo + 1, :]
                    )
                    nc.tensor.matmul(
                        PT_ps[p][:],
                        lhsT=pars[s][:, p, :],
                        rhs=lmat_s,
                        start=False,
                        stop=s == nsub - 1,
                    )

            # ---- sort projections (per pair), batched softmax (both pairs)
            PT_sb = [
                small.tile([64, 128], FP, tag=f"PTs{p}", name=f"PT_sb{p}")
                for p in range(NPAIR)
            ]
            FT_sb = [
                small.tile([64, 128], FP, tag=f"FTs{p}", name=f"FT_sb{p}")
                for p in range(NPAIR)
            ]
            for p in range(NPAIR):
                nc.scalar.copy(FT_sb[p][:], FT_ps[p][:])
                nc.scalar.copy(PT_sb[p][:], PT_ps[p][:])
            if debug_taps:
                for p in range(NPAIR):
                    nc.sync.dma_start(taps["pt"][64 * p : 64 * p + 64], PT_sb[p][:])
                    nc.sync.dma_start(taps["ft"][64 * p : 64 * p + 64], FT_sb[p][:])


            # SKQ rows: 0:40 sort-q blocks (b0 at 0:8, b1 at 32:40),
            #           64:104 sort-k blocks (b0 at 64:72, b1 at 96:104);
            # one PSUM bank per pair: each holds a long-open accumulation group
            # opened by the constant-term matmul (ready at kernel start) and
            # closed by the PT-part matmul (the only one on the critical tail)
            C104O = NC128
            SQs = []
            RKs = []
            for p in range(NPAIR):
                sk_ps_t = pp.tile([104, 128], FP, tag=f"SKQ{p}")
                sk_ps = sk_ps_t[:]
                nc.tensor.matmul(
                    sk_ps,
                    lhsT=ident[0:104, 0:104],
                    rhs=cp_sb[0:104, C104O + 128 * p : C104O + 128 * p + 128],
                    start=True,
                    stop=False,
                    skip_group_check=True,
                )
                nc.tensor.matmul(
                    sk_ps,
                    lhsT=c64_sb[:, 208 + 104 * p : 312 + 104 * p],
                    rhs=FT_sb[p][:],
                    start=False,
                    stop=False,
                    skip_group_check=True,
                )
                nc.tensor.matmul(
                    sk_ps,
                    lhsT=c64_sb[:, 104 * p : 104 * p + 104],
                    rhs=PT_sb[p][:],
                    start=False,
                    stop=True,
                    skip_group_check=True,
                )
                sq_sb = small.tile([40, 128], FP, tag=f"SQ{p}")
                nc.scalar.copy(sq_sb[:], sk_ps[0:40, :])
                rk_sb = small.tile([40, 128], FP, tag=f"RK{p}")
                nc.vector.tensor_copy(rk_sb[:], sk_ps[64:104, :])
                SQs.append(sq_sb)
                RKs.append(rk_sb)

            # R group: opened early by an identity-weighted matmul that seeds
            # the bank with the additive causal mask; the four sq.sk matmuls
            # then accumulate into their quadrants, so the masked logits sit
            # in PSUM with no extra elementwise pass
            R_ps = pp.tile([128, 128], FP, tag="R")
            nc.tensor.matmul(
                R_ps[:],
                lhsT=ident,
                rhs=cp_sb[:, 384:512],
                start=True,
                stop=False,
                skip_group_check=True,
            )
            for p in range(NPAIR):
                nc.tensor.matmul(
                    R_ps[0:64, 64 * p : 64 * p + 64],
                    lhsT=SQs[p][0:8, 0:64],
                    rhs=RKs[p][0:8, 0:64],
                    start=False,
                    stop=False,
                    skip_group_check=True,
                )
                nc.tensor.matmul(
                    R_ps[64:128, 64 * p : 64 * p + 64],
                    lhsT=SQs[p][32:40, 64:128],
                    rhs=RKs[p][32:40, 64:128],
                    start=False,
                    stop=p == NPAIR - 1,
                    skip_group_check=True,
                )

            # masked softmax over 65 logits (implicit zero-logit column 0),
            # both pairs batched along the free axis: cols = (pair, j)
            Rm = R_ps[:].rearrange("q (p j) -> q p j", p=2)
            mx = small.tile([128, 2], FP, tag="mx")
            nc.vector.reduce_max(mx[:], Rm, axis=X)
            negm = small.tile([128, 2], FP, tag="negm")
            nc.vector.tensor_scalar(
                negm[:], mx[:], 0.0, -1.0,
                op0=mybir.AluOpType.max, op1=MULT,
            )
            e0 = small.tile([128, 2], FP, tag="e0")
            nc.scalar.activation(e0[:], negm[:], Exp)
            e_sb = small.tile([128, 2, 64], FP, tag="e")
            for p in range(NPAIR):
                nc.scalar.activation(
                    e_sb[:, p, :], R_ps[:, 64 * p : 64 * p + 64], Exp,
                    bias=negm[:, p : p + 1], scale=1.0,
                )
            s1 = small.tile([128, 2], FP, tag="s1")
            nc.vector.reduce_sum(s1[:], e_sb[:], axis=X)
            den = small.tile([128, 2], FP, tag="den")
            nc.vector.tensor_add(den[:], s1[:], e0[:])
            rin = small.tile([128, 2], FP, tag="rin")
            nc.vector.reciprocal(rin[:], den[:])
            outt = small.tile([128, 2, BUCKETS + 1], FP, tag="outt")
            for p in range(NPAIR):
                # outt = (e * 1/den) * tril-mask, fused
                nc.vector.scalar_tensor_tensor(
                    outt[:, p, 1:],
                    e_sb[:, p, :],
                    rin[:, p : p + 1],
                    mmask_b[:, p, :],
                    op0=MULT,
                    op1=MULT,
                )
            t0 = small.tile([128, 2], FP, tag="t0")
            nc.vector.tensor_mul(t0[:], e0[:], mask0_b)
            nc.vector.tensor_mul(outt[:, :, 0], t0[:], rin[:])
            nc.sync.dma_start(
                rout.rearrange("(p b) i c -> (b i) p c", p=2), outt[:]
            )

    nc.compile()
    return nc


def _get_program(t_seq=T, enable_asserts=False):
    key = (t_seq, enable_asserts)
    if key not in _PROG_CACHE:
        _PROG_CACHE[key] = _build_program(t_seq, enable_asserts=enable_asserts)
    return _PROG_CACHE[key]


def _host_constants(core, q_pos_emb, k_pos_emb, Wsq, Wsk, chunk=CHUNK):
    """Single packed per-core constant tensor."""
    f32 = np.float32
    j = np.arange(64, dtype=np.float64)
    s = (1.0 / (chunk * j + 1.0)).astype(f32)  # per-bucket cumavg scale

    tri = np.triu(np.ones((64, 64), f32), k=1)  # [c, j] = 1 iff c < j
    tri_s = tri * s[None, :]
    lmat_s = np.zeros((128, 128), f32)
    lmat_s[0:64, 0:64] = tri_s
    lmat_s[64:128, 64:128] = tri_s
    idents = np.zeros((128, 128), f32)
    idents[np.arange(128), np.arange(128)] = np.concatenate([s, s])
    ident = np.eye(128, dtype=f32)

    rows = np.arange(64)[:, None]
    cols = np.arange(64)[None, :]
    am = np.where(cols < rows, 0.0, NEG).astype(f32)       # softmax additive mask
    mm = (cols <= rows - 2).astype(f32)                    # output tril(-1) mask
    amask_b = np.concatenate([am, am], axis=1)
    amask_b = np.concatenate([amask_b, amask_b], axis=0)
    mmask_b = np.concatenate([mm, mm], axis=1)
    mmask_b = np.concatenate([mmask_b, mmask_b], axis=0)
    m0 = (np.arange(64) > 0).astype(f32).reshape(64, 1)
    mask0_b = np.concatenate([np.concatenate([m0, m0], 1)] * 2, 0)

    c128 = np.concatenate([lmat_s, idents, ident, amask_b, mmask_b, mask0_b], axis=1)

    wq_pt = np.zeros((2, 64, 104), f32)   # [pair][d][sq 0:40 | sk 64:104]
    wq_ft = np.zeros((2, 64, 104), f32)
    cblk = np.zeros((2, 104, 128), f32)   # [pair][skq-row][(b, j)]
    for p in range(NPAIR):
        for b in range(2):
            bh = core * BHC + 2 * p + b
            h = bh % HEADS
            r0 = 32 * b
            wq_pt[p, :, r0 : r0 + 8] = Wsq[0, h, 0:64, :]
            wq_pt[p, :, 64 + r0 : 64 + r0 + 8] = Wsk[0, h, 0:64, :]
            wq_ft[p, :, r0 : r0 + 8] = Wsq[0, h, 64:128, :]
            wq_ft[p, :, 64 + r0 : 64 + r0 + 8] = Wsk[0, h, 64:128, :]
            cq = q_pos_emb[0, h] @ Wsq[0, h, 128:192, :]  # (64, 8)
            ck = k_pos_emb[0, h] @ Wsk[0, h, 128:192, :]
            cblk[p, r0 : r0 + 8, 64 * b : 64 * b + 64] = cq.T
            cblk[p, 64 + r0 : 64 + r0 + 8, 64 * b : 64 * b + 64] = ck.T

    c64 = np.concatenate([wq_pt[0], wq_pt[1], wq_ft[0], wq_ft[1]], axis=1)
    c104 = np.concatenate([cblk[0], cblk[1]], axis=1)
    c104 = np.concatenate([c104, np.zeros((24, NC104), f32)], axis=0)
    cpack = np.concatenate([c128, c104], axis=1)
    assert cpack.shape == (128, NCALL), cpack.shape
    assert c64.shape == (64, NC64), c64.shape
    return {"cpack": cpack, "c64": c64}


def _run(k, q_pos_emb, k_pos_emb, Wsq, Wsk, trace=False, t_seq=T):
    nc = _get_program(t_seq)
    in_maps = []
    for core in range(NCORES):
        cm = _host_constants(
            core, q_pos_emb, k_pos_emb, Wsq, Wsk, chunk=t_seq // BUCKETS
        )
        cm["kin"] = np.ascontiguousarray(k[core * BHC : (core + 1) * BHC])
        in_maps.append(cm)
    res = bass_utils.run_bass_kernel_spmd(
        nc,
        in_maps,
        core_ids=list(range(NCORES)),
        trace=trace,
        **(TRACE_KWARGS if trace else {}),
    )
    global LAST_RESULTS
    LAST_RESULTS = res
    out = np.concatenate([r["rout"] for r in res.results], axis=0)
    return out, res


def kernel(**inputs):
    k = np.asarray(inputs["k"], np.float32)
    q_pos_emb = np.asarray(inputs["q_pos_emb"], np.float32)
    k_pos_emb = np.asarray(inputs["k_pos_emb"], np.float32)
    Wsq = np.asarray(inputs["Wsq"], np.float32)
    Wsk = np.asarray(inputs["Wsk"], np.float32)
    out, _ = _run(k, q_pos_emb, k_pos_emb, Wsq, Wsk, trace=TRACE)
    return out
